# revision 29
# baseline (speedup 1.0000x reference)
"""BlockGRUCell Trainium2 kernel.

Computation (per reference):
  hx = concat([h, x], -1)                       # (B, 2048)
  gate[b, 192g+o] = sum_i hx[b, 128g+i] * W[g, o, i]   # block-diagonal matmul
  r, c, u = split(gate + bias, 3)               # bias == 0 from setup_inputs
  h_new = sigmoid(u) * tanh(sigmoid(r) * c) + (1 - sigmoid(u)) * h

Sharding: data-parallel over batch across 8 NeuronCores (2048 rows each),
weights replicated.

The TensorE matmul contracts over the partition dim, so the stationary
operand must be hx^T per 128-feature block. The host pre-packs hx into
per-tile transposed fp8(e4m3) panels:
  hxt[t, p, 128g+b] = hx[128t+b, 128g+p]

Engine layout (per core, measured):
  - ACT is the binding engine in steady state (~6.1us per tile pair):
    3 LUT passes per element are irreducible; sigmoid(r)/sigmoid(u) read
    f32 PSUM panels per tile at spec (997ns; the 8-bank PSUM ring cannot
    hold pairs), tanh runs PAIR-wide from SBUF (FD=2048).  Wider (quad)
    ops or fine-grained ramp splits head-of-line block the in-order
    engine queues and measurably regress.
  - DVE: rc = gC(PSUM f32)*reset runs 1x; the blend chain runs
    pair-wide in bf16 (2x_1P): dd2 = cand2-h2, ee2 = upd2*dd2,
    out2 = h2+ee2 -> 5 DVE ops per pair (~6.0us/pair, just under ACT).
  - h / out move as bf16 pairs (tolerance 2e-2; bf16 adds ~3e-3,
    fp8 hx adds ~5e-3).
"""

import numpy as np
import ml_dtypes

import concourse.bass as bass
import concourse.bacc as bacc
import concourse.tile as tile
import concourse.mybir as mybir
from concourse.bass_utils import run_bass_kernel_spmd

N_CORES = 8
BATCH = 16384
BS = BATCH // N_CORES            # rows per core
P = 128
NT = BS // P                     # 128-row tiles per core
HID = 1024
G = 16                           # feature blocks
IN_PER = 128
OUT_PER = 192
GATE = 3 * HID                   # 3072
PSUM_BANK_F32 = 512

F32 = mybir.dt.float32
BF16 = mybir.dt.bfloat16
FP8 = mybir.dt.float8e4
AFT = mybir.ActivationFunctionType

HXT_FP8 = True                   # hx^T panels in fp8 e4m3 (halves hxt DMA)
HXT_DT = FP8 if HXT_FP8 else BF16
HXT_NP = ml_dtypes.float8_e4m3 if HXT_FP8 else ml_dtypes.bfloat16


def _mm_splits(block_major):
    """[(c0, c1, g)] matmul column splits at PSUM bank boundaries."""
    out = []
    for g in range(G):
        c0 = g * OUT_PER
        while c0 < (g + 1) * OUT_PER:
            c1 = min((g + 1) * OUT_PER,
                     (c0 // PSUM_BANK_F32 + 1) * PSUM_BANK_F32)
            out.append((c0, c1, g))
            c0 = c1
    if not block_major:
        out.sort(key=lambda s: s[0])
    return out


def _body(tc, nc, hxt_d, h_d, wt_d, out_d):
    with (
        tc.tile_pool(name="consts", bufs=1) as consts,
        tc.tile_pool(name="io", bufs=6) as io,
        tc.tile_pool(name="panels", bufs=4) as panels,
        tc.tile_pool(name="pairs", bufs=3) as pairs,
        tc.tile_pool(name="gatep", bufs=4, space="PSUM") as gatep,
    ):
        # warm the sigmoid/tanh ACT table during the initial DMAs (the
        # ~2.7us ACT_TABLE_LOAD otherwise lands on tile 0's critical path)
        warm = consts.tile([P, 1], F32)
        nc.vector.memset(warm, 0.0)
        nc.scalar.activation(warm, warm, AFT.Sigmoid)

        # split the weight load so tile 0's r-gate matmuls start sooner
        # (both a 3-way split and a contiguous-repacked 2-way split were
        # tried and regress ~13us via scheduler butterfly effects)
        wt_s = consts.tile([P, G * OUT_PER], BF16)
        nc.sync.dma_start(out=wt_s[:, 0:GATE // 2], in_=wt_d[:, 0:GATE // 2])
        nc.sync.dma_start(out=wt_s[:, GATE // 2:], in_=wt_d[:, GATE // 2:])

        h2 = None
        out2 = None
        rc2 = None
        upd2 = None
        for t in range(NT):
            q, half = divmod(t, 2)
            last_pair = q == NT // 2 - 1
            # ramp lookahead: cap early hxt prefetch at 4 tiles so the
            # first tiles' critical DMAs aren't starved for bandwidth
            hxt = io.tile([P, G * P], HXT_DT, tag="hxt", bufs=5)
            if t == 0:
                nc.sync.dma_start(out=hxt[:, 0:G * P // 2],
                                  in_=hxt_d[0, :, 0:G * P // 2])
                nc.sync.dma_start(out=hxt[:, G * P // 2:],
                                  in_=hxt_d[0, :, G * P // 2:])
            else:
                nc.sync.dma_start(out=hxt, in_=hxt_d[t])
            if half == 0:
                # h arrives pair-packed in bf16: one 512K DMA per two tiles.
                # For the first pair, defer the load until after the matmul
                # feeds so it doesn't compete with the critical-path DMAs.
                h2 = io.tile([P, 2 * HID], BF16, tag="h2", bufs=2)
                if t > 0:
                    nc.sync.dma_start(out=h2, in_=h_d[q])
                out2 = io.tile([P, 2 * HID], BF16, tag="out2", bufs=3)
                rc2 = pairs.tile([P, 2 * HID], BF16, tag="rc2")
                upd2 = pairs.tile([P, 2 * HID], BF16, tag="upd2")

            # gate panels = the r/c/u split exactly (2 PSUM banks each)
            gR = gatep.tile([P, HID], F32, tag="gate")
            gC = gatep.tile([P, HID], F32, tag="gate")
            gU = gatep.tile([P, HID], F32, tag="gate")
            gs = (gR, gC, gU)

            for c0, c1, g in _mm_splits(block_major=True):
                gate = gs[c0 // HID]
                nc.tensor.matmul(gate[:, c0 % HID:(c0 % HID) + c1 - c0],
                                 hxt[:, g * P:(g + 1) * P], wt_s[:, c0:c1],
                                 start=True, stop=True)

            if t == 0:
                nc.sync.dma_start(out=h2, in_=h_d[0])

            reset = panels.tile([P, HID], F32, tag="reset")
            rc_t = rc2[:, half * HID:(half + 1) * HID]
            upd_t = upd2[:, half * HID:(half + 1) * HID]

            if not last_pair:
                nc.scalar.activation(reset, gR, AFT.Sigmoid)
                nc.scalar.activation(upd_t, gU, AFT.Sigmoid)
                nc.vector.tensor_tensor(rc_t, gC, reset,
                                        mybir.AluOpType.mult)
                if half == 1:
                    # pair-wide epilogue: FD=2048 amortizes the per-op bubble
                    cand2 = pairs.tile([P, 2 * HID], BF16, tag="cand2")
                    dd2 = pairs.tile([P, 2 * HID], BF16, tag="dd2")
                    ee2 = pairs.tile([P, 2 * HID], BF16, tag="ee2")
                    nc.scalar.activation(cand2, rc2, AFT.Tanh)
                    nc.vector.tensor_sub(dd2, cand2, h2)
                    nc.vector.tensor_mul(ee2, upd2, dd2)
                    nc.vector.tensor_add(out2, h2, ee2)
                    nc.sync.dma_start(out=out_d[q], in_=out2)
            else:
                # tail: run per-tile (halves, then quarters on the final
                # tile) so the serial ACT<->DVE chain drains fine-grained
                # and the final stores stream out early
                cand = panels.tile([P, HID], BF16, tag="cand")
                dd = panels.tile([P, HID], BF16, tag="dd")
                ee = panels.tile([P, HID], BF16, tag="ee")
                hn = out2[:, half * HID:(half + 1) * HID]
                splits = [(0, HID // 2), (HID // 2, HID)] if half == 0 else \
                         [(k * HID // 4, (k + 1) * HID // 4)
                          for k in range(4)]
                # phase 1: drain the PSUM panels (keeps ACT's in-order
                # queue free of tanh ops that wait on DVE)
                for a, b in splits:
                    nc.scalar.activation(reset[:, a:b], gR[:, a:b],
                                         AFT.Sigmoid)
                    nc.vector.tensor_tensor(rc_t[:, a:b], gC[:, a:b],
                                            reset[:, a:b],
                                            mybir.AluOpType.mult)
                    nc.scalar.activation(upd_t[:, a:b], gU[:, a:b],
                                         AFT.Sigmoid)
                # phase 2: tanh + blend + streaming stores
                for idx, (a, b) in enumerate(splits):
                    nc.scalar.activation(cand[:, a:b], rc_t[:, a:b],
                                         AFT.Tanh)
                    nc.vector.tensor_sub(dd[:, a:b], cand[:, a:b],
                                         h2[:, half * HID + a:
                                            half * HID + b])
                    nc.vector.tensor_mul(ee[:, a:b], upd_t[:, a:b],
                                         dd[:, a:b])
                    nc.vector.tensor_add(hn[:, a:b],
                                         h2[:, half * HID + a:
                                            half * HID + b], ee[:, a:b])
                    if half == 1:
                        lo = HID + a
                        nc.sync.dma_start(out=out_d[q][:, lo:HID + b],
                                          in_=out2[:, lo:HID + b])
                if half == 0:
                    nc.sync.dma_start(out=out_d[q][:, 0:HID],
                                      in_=out2[:, 0:HID])


_NC_CACHE = {}


def _build_nc():
    if "nc" in _NC_CACHE:
        return _NC_CACHE["nc"]
    nc = bacc.Bacc()
    hxt_d = nc.dram_tensor("hxt", [NT, P, G * P], HXT_DT, kind="ExternalInput")
    h_d = nc.dram_tensor("h2", [NT // 2, P, 2 * HID], BF16,
                         kind="ExternalInput")
    wt_d = nc.dram_tensor("wt", [P, G * OUT_PER], BF16, kind="ExternalInput")
    out_d = nc.dram_tensor("out", [NT // 2, P, 2 * HID], BF16,
                           kind="ExternalOutput")
    with tile.TileContext(nc) as tc:
        _body(tc, nc, hxt_d, h_d, wt_d, out_d)
    nc.compile()
    _NC_CACHE["nc"] = nc
    return nc


def _np_reference(x, h, weight, bias):
    hx = np.concatenate([h, x], axis=-1)
    xg = hx.reshape(x.shape[0], G, IN_PER)
    gate = np.einsum("bgi,goi->bgo", xg, weight).reshape(x.shape[0], GATE)
    gate = gate + bias
    r, c, u = np.split(gate, 3, axis=-1)
    reset = 1.0 / (1.0 + np.exp(-r))
    cand = np.tanh(reset * c)
    upd = 1.0 / (1.0 + np.exp(-u))
    return (upd * cand + (1.0 - upd) * h).astype(np.float32)


def _pack_hxt(hs, xs):
    """-> [NT, 128, 2048] with hxt[t, p, 128g+b] = hx[128t+b, 128g+p],
    where hx = concat([h, x], -1) per-row (blocks 0-7 = h, 8-15 = x)."""
    def tp(a):                      # [BS, 1024] -> [NT, 128, 8, 128]
        return a.reshape(NT, P, 8, P).transpose(0, 3, 2, 1)   # [t, p, g, b]
    arr = np.concatenate([tp(hs), tp(xs)], axis=2)            # [t, p, 16, b]
    return np.ascontiguousarray(arr.reshape(NT, P, G * P)).astype(HXT_NP)


def _pack_pairs(a):
    """[BS, 1024] -> [NT//2, 128, 2048] bf16 with
    [q, p, 1024s+f] = a[256q+128s+p, f]."""
    return np.ascontiguousarray(
        a.reshape(NT // 2, 2, P, HID).transpose(0, 2, 1, 3)
        .reshape(NT // 2, P, 2 * HID)).astype(ml_dtypes.bfloat16)


def _unpack_pairs(a):
    """inverse of _pack_pairs, upcast to fp32."""
    return np.ascontiguousarray(
        a.reshape(NT // 2, P, 2, HID).transpose(0, 2, 1, 3)
        .reshape(BS, HID)).astype(np.float32)


def _run(x, h, weight, bias, trace=False, tmpdir=None):
    # wt[p, 192g+o] = W[g, o, p] — the exact SBUF layout, one contiguous DMA
    wt = np.ascontiguousarray(
        weight.transpose(2, 0, 1).reshape(P, G * OUT_PER)).astype(
        ml_dtypes.bfloat16)
    nc = _build_nc()
    in_maps = []
    for c in range(N_CORES):
        sl = slice(c * BS, (c + 1) * BS)
        xs, hs = x[sl], h[sl]
        in_maps.append({
            "hxt": _pack_hxt(hs, xs),
            "h2": _pack_pairs(hs),
            "wt": wt,
        })
    res = run_bass_kernel_spmd(nc, in_maps, core_ids=list(range(N_CORES)),
                               trace=trace, tmpdir=tmpdir)
    out = np.concatenate([_unpack_pairs(m["out"]) for m in res.results],
                         axis=0)
    return out, res


def kernel(x, h, weight, bias):
    x = np.asarray(x, dtype=np.float32)
    h = np.asarray(h, dtype=np.float32)
    weight = np.asarray(weight, dtype=np.float32)
    bias = np.asarray(bias, dtype=np.float32)
    if np.any(bias != 0.0):
        # setup_inputs() always passes zero bias; keep a correct fallback.
        return _np_reference(x, h, weight, bias)
    out, _ = _run(x, h, weight, bias)
    return out
